# revision 4
# baseline (speedup 1.0000x reference)
"""Distributed Trainium2 (Bass/Tile) kernel for masked GAT-style attention.

Reference computation (H=4 heads, N=4096 nodes, D=128):
    scores = leaky_relu(x @ W^T + b, 0.2)            # [H, N, N]
    att    = where(mask, softmax(where(mask, scores, -inf)), 0)
    out    = att @ x                                  # [H, N, D]

Sharding: 8 cores = 4 heads x 2 row-blocks of 2048 nodes. Each core
computes out[h, r0:r0+2048] independently (no collectives).

Per-core layout ("transposed scores"): scores^T tiles [m=128 part, n free]
so the PV matmul uses the attention tile directly as the stationary
operand and the softmax row-sum comes for free from an appended
ones-column on x. Elementwise work (leaky_relu via Prelu LUT, exp) runs
on the Scalar engine; the mask multiply runs on the Vector engine in
bf16 (2x mode).
"""

import sys

if "/opt/trn_rl_repo" not in sys.path:
    sys.path.insert(0, "/opt/trn_rl_repo")

import numpy as np
import ml_dtypes

import concourse.bass as bass
import concourse.tile as tile
from concourse import bacc, mybir
from concourse.bass_utils import run_bass_kernel_spmd

BF = mybir.dt.bfloat16
F32 = mybir.dt.float32
BF_NP = ml_dtypes.bfloat16

H, N, D = 4, 4096, 128
N_CORES = 8
ROWS = N * H // N_CORES          # 2048 rows (n) per core
CHUNK = 1024                     # n columns processed per outer chunk
CHUNKS = ROWS // CHUNK           # 2
M_TILES = N // 128               # 32 tiles along the softmax (m) axis
SUBS = CHUNK // 128              # 8 PV subtiles per chunk

# Module-level knobs used by test.py; harmless defaults for grading.
TRACE = False
LAST_EXEC_NS = None

_CACHED_NC = None


def _build_nc():
    nc = bacc.Bacc("TRN2", target_bir_lowering=False, debug=False,
                   num_devices=N_CORES)
    xt_d = nc.dram_tensor("xt", [128, ROWS], BF, kind="ExternalInput").ap()
    wt_d = nc.dram_tensor("wt", [128, N], BF, kind="ExternalInput").ap()
    xa_d = nc.dram_tensor("xa", [N, D + 1], BF, kind="ExternalInput").ap()
    mk_d = nc.dram_tensor("mk", [N, ROWS], BF, kind="ExternalInput").ap()
    bc_d = nc.dram_tensor("bc", [128, M_TILES], F32, kind="ExternalInput").ap()
    out_d = nc.dram_tensor("out", [ROWS, D], F32, kind="ExternalOutput").ap()

    PRELU = mybir.ActivationFunctionType.Prelu
    EXP = mybir.ActivationFunctionType.Exp

    with tile.TileContext(nc) as tc:
        with (
            tc.tile_pool(name="const", bufs=1) as cpool,
            tc.tile_pool(name="mask", bufs=3) as mpool,
            tc.tile_pool(name="work", bufs=3) as wpool,
            tc.tile_pool(name="outp", bufs=3) as opool,
            tc.tile_pool(name="spsum", bufs=2, space="PSUM") as spool,
            tc.tile_pool(name="opsum", bufs=1, space="PSUM") as oppool,
        ):
            wt_sb = cpool.tile([128, N], BF)
            nc.sync.dma_start(out=wt_sb[:], in_=wt_d[:, :])
            xt_sb = cpool.tile([128, ROWS], BF)
            nc.sync.dma_start(out=xt_sb[:], in_=xt_d[:, :])
            bc_sb = cpool.tile([128, M_TILES], F32)
            nc.sync.dma_start(out=bc_sb[:], in_=bc_d[:, :])
            xa_sb = cpool.tile([128, M_TILES, D + 1], BF)
            for m in range(M_TILES):
                nc.sync.dma_start(
                    out=xa_sb[:, m], in_=xa_d[m * 128:(m + 1) * 128, :]
                )

            for c in range(CHUNKS):
                # 8 accumulator subtiles of [128, 129] packed 3-per-bank.
                o_ps = [
                    oppool.tile([128, 512], F32, tag=f"oacc{b}",
                                name=f"oacc{b}_c{c}")
                    for b in range((SUBS + 2) // 3)
                ]

                def o_ap(s):
                    return o_ps[s // 3][:, (s % 3) * 129:(s % 3) * 129 + 129]

                for m in range(M_TILES):
                    # scores^T tile: [m=128, n=CHUNK] (two 512-col matmuls,
                    # one per PSUM bank).
                    s_ps = spool.tile([128, CHUNK], F32)
                    for half in range(CHUNK // 512):
                        nc.tensor.matmul(
                            s_ps[:, half * 512:(half + 1) * 512],
                            lhsT=wt_sb[:, m * 128:(m + 1) * 128],
                            rhs=xt_sb[:, c * CHUNK + half * 512:
                                      c * CHUNK + (half + 1) * 512],
                            start=True, stop=True,
                        )
                    # leaky_relu(s + b) on ScalarE (Prelu LUT, alpha=0.2),
                    # bias is per-partition = b[h] along m.
                    l_sb = wpool.tile([128, CHUNK], F32, tag="l")
                    nc.scalar.activation(l_sb[:], s_ps[:], PRELU,
                                         bias=bc_sb[:, m:m + 1], scale=1.0,
                                         alpha=0.2)
                    e_sb = wpool.tile([128, CHUNK], BF, tag="e")
                    nc.scalar.activation(e_sb[:], l_sb[:], EXP)

                    mk_sb = mpool.tile([128, CHUNK], BF)
                    nc.sync.dma_start(
                        out=mk_sb[:],
                        in_=mk_d[m * 128:(m + 1) * 128,
                                 c * CHUNK:(c + 1) * CHUNK],
                    )
                    a_sb = wpool.tile([128, CHUNK], BF, tag="a")
                    nc.vector.tensor_mul(a_sb[:], e_sb[:], mk_sb[:])

                    # PV: out[n, 0:128] += att^T.T @ x ; col 128 = row-sum.
                    # start=True clears has_written for the WHOLE bank, so
                    # only the first sub-chain of each bank may issue it;
                    # later sub-ranges land via per-element overwrite-on-
                    # first-write semantics.
                    for s in range(SUBS):
                        nc.tensor.matmul(
                            o_ap(s),
                            lhsT=a_sb[:, s * 128:(s + 1) * 128],
                            rhs=xa_sb[:, m],
                            start=(m == 0 and s % 3 == 0),
                            stop=(m == M_TILES - 1),
                            skip_group_check=True,
                        )

                for s in range(SUBS):
                    ob = o_ap(s)
                    r_sb = opool.tile([128, 1], F32, tag="recip")
                    nc.vector.reciprocal(r_sb[:], ob[:, 128:129])
                    of_sb = opool.tile([128, D], F32, tag="of")
                    nc.vector.tensor_scalar_mul(of_sb[:], ob[:, 0:D], r_sb[:])
                    row = c * CHUNK + s * 128
                    nc.sync.dma_start(out=out_d[row:row + 128, :], in_=of_sb[:])

    nc.compile()
    return nc


def kernel(x, W, b, neighbor_mask):
    global _CACHED_NC, LAST_EXEC_NS
    x = np.asarray(x, dtype=np.float32)
    W = np.asarray(W, dtype=np.float32)
    b = np.asarray(b, dtype=np.float32)
    mask = np.asarray(neighbor_mask)

    if _CACHED_NC is None:
        _CACHED_NC = _build_nc()
    nc = _CACHED_NC

    mask_bf = mask.astype(BF_NP)
    in_maps = []
    for core in range(N_CORES):
        h, rb = core // 2, core % 2
        r0 = rb * ROWS
        xt = np.ascontiguousarray(x[h, r0:r0 + ROWS].T).astype(BF_NP)
        wt = np.ascontiguousarray(W[h].T).astype(BF_NP)
        xa = np.concatenate(
            [x[h], np.ones((N, 1), np.float32)], axis=1
        ).astype(BF_NP)
        mk = np.ascontiguousarray(mask_bf[r0:r0 + ROWS].T)
        bc = np.ascontiguousarray(b[h].reshape(M_TILES, 128).T)
        in_maps.append({"xt": xt, "wt": wt, "xa": xa, "mk": mk, "bc": bc})

    res = run_bass_kernel_spmd(nc, in_maps, core_ids=list(range(N_CORES)),
                               trace=TRACE)
    LAST_EXEC_NS = res.exec_time_ns

    out = np.empty((H, N, D), np.float32)
    for core in range(N_CORES):
        h, rb = core // 2, core % 2
        r0 = rb * ROWS
        out[h, r0:r0 + ROWS] = res.results[core]["out"]
    return out
